# revision 17
# baseline (speedup 1.0000x reference)
"""Balanced focal NT-Xent loss on 8 TRN2 NeuronCores — symmetric half-matrix.

The 8192x8192 similarity matrix S = zn zn^T is symmetric, so exp(2*S) is
computed ONCE per unordered 512x512 block pair. With 16 row-blocks of 512,
core c owns row-blocks A=c (col offsets +0..+8) and B=c+8 (offsets +0..+7):
exactly 17 of the 136 unique blocks per core, perfectly balanced, and the
positive-pair block (c, c+8) lands on core c's A strip.

Inputs are host-normalized (zn = z/|z| in fp32), quantized to fp8 e4m3
(halves DMA/SBUF; final rel err ~1e-5 vs the 2e-2 budget) and
column-rotated by -512c per core so every core runs one static SPMD
program over contiguous column spans: A covers [0:4608), B [4096:8192),
lhsT at [0:512) / [4096:4608). Both 128-row contraction chunks sit side by
side in one [128, 4, 2, 2048] tile so a single DoubleRow fp8 matmul
contracts all 256 rows — the kernel is then paced purely by the scalar
engine's fused exp (1 elem/cycle/lane at 1.2 GHz over 4.46M elements/core).

v2 changes vs baseline:
- Input arrives as ONE host-packed DRAM tensor matching the SBUF layout,
  loaded by 4 ordered HWDGE DMAs on the sync queue (block0's first 1536
  cols land first so span 0's matmuls can start ~2us in, vs ~11us when 12
  column-strided dma_starts fought over descriptor-gen slots). A small aux
  tensor carries a host-built I128 + one-hot column (bf16 bitcast in fp8
  bytes), killing the on-chip memsets.
- Column-sum add-tree runs once per SPAN at full width (15 DVE adds
  instead of 39 block-wise ones) — fewer instructions, fewer semaphores,
  shorter end-of-kernel semaphore-clear epilogue.
- Row-sum slots are PE-transposed ([128,24] -> [24,128]) and both outputs
  leave in one [24, 640] DMA with >=512B descriptors (the [128, 96B]
  output swarm cost ~1.8us in the baseline tail).

Row sums come free from the exp's accum_out; column sums (the transpose
side of each off-diagonal block) use the DVE add-tree (4 row-tiles -> 1,
bf16) plus a one-hot ones-matmul accumulating every block into one
[16, 512] PSUM bank (the final span skips the tree and accumulates its E
row-tiles directly so the post-exp tail stays short).

The host combines the per-core partial sums (O(N) numpy): S scatter-add,
self/positive dots, ce = ln(S - exp(2|q|^2)) - 2*pos, focal, mean.
"""

import sys

if "/opt/trn_rl_repo" not in sys.path:
    sys.path.insert(0, "/opt/trn_rl_repo")

import numpy as np
import ml_dtypes

import concourse.tile as tile
from concourse import bacc, mybir
from concourse.bass_utils import run_bass_kernel_spmd

B = 4096
D = 256
N = 2 * B          # 8192
NCORES = 8
BLK = 512          # symmetric block size
NBLK = N // BLK    # 16
TEMPERATURE = 0.5
GAMMA = 2.0
ALPHA = 0.25

BF16 = mybir.dt.bfloat16
FP8 = mybir.dt.float8e4
F32 = mybir.dt.float32

DBLK = 2048        # DMA column block
SW = 1536          # span tile width (3 PSUM banks)
AUXW = 320         # aux bytes/partition: I128 bf16 (256B) + one-hot col (64B)

# (col_lo, width, lhs_dma_block) for the span strips; spans 0-3 are the
# A row-block (own cols at [0:512)), spans 4-6 the B row-block ([4096:4608)).
# The first two spans are narrow so the first EXP gates on a 128KB DMA
# piece instead of 384KB — the exp train starts ~2.5us earlier, worth the
# +4 activations of fixed cost.
SPANS = [
    (0, 512, 0),
    (512, 1024, 0),
    (1536, 1536, 0),
    (3072, 1536, 0),
    (4096, 1536, 2),
    (5632, 1536, 2),
    (7168, 1024, 2),
]
NSPAN = len(SPANS)
# diag blocks (rowsum-only): rotated block 0 (A diag, in span 0) and
# block 8 (B diag, in span 4)
SKIP_COLSUM = {0: (0,), 4: (8,)}
NCS_TOT = 10 + 12 + 8  # 10 tree-reduced + 20 direct in the last two spans


def build_nc():
    nc = bacc.Bacc(None, target_bir_lowering=False)
    zin = nc.dram_tensor("zin", [128, N // DBLK, 2, DBLK], FP8,
                         kind="ExternalInput")
    aux = nc.dram_tensor("aux", [128, AUXW], FP8, kind="ExternalInput")
    out_cs = nc.dram_tensor("out_cs", [16, BLK], F32, kind="ExternalOutput")
    out_st = nc.dram_tensor("out_st", [4 * NSPAN, 128], F32,
                            kind="ExternalOutput")

    with tile.TileContext(nc) as tc:
        with (
            tc.tile_pool(name="big", bufs=1) as big,
            tc.tile_pool(name="epool", bufs=3) as epool,
            tc.tile_pool(name="ps", bufs=2, space="PSUM") as ps,
        ):
            # input z^T (normalized, rotated): 4 col-blocks, each holding
            # both 128-row contraction chunks side by side so a single
            # DoubleRow fp8 matmul contracts all 256 rows.
            zbig = big.tile([128, N // DBLK, 2, DBLK], FP8, tag="zbig",
                            name="zbig")
            auxt = big.tile([128, AUXW], FP8, tag="aux", name="auxt")
            auxb = auxt.bitcast(BF16)  # [:,0:128]=I128, [:,128:160]=one-hot
            slots = epool.tile([128, 4 * NSPAN], F32, tag="slots", bufs=1)
            slots_b = epool.tile([128, 4 * NSPAN], BF16, tag="slots_b", bufs=1)
            outcs = epool.tile([16, BLK], F32, tag="outcs", bufs=1)
            outst = epool.tile([4 * NSPAN, 128], F32, tag="outst", bufs=1)

            # All input DMAs ride the sync HWDGE ring, whose execution is
            # FIFO: pieces land in exactly consumption order, and the first
            # piece is the minimal 128KB span 0 needs (both rings feed the
            # same 16 SDMA engines round-robin, so a second ring gives
            # bandwidth sharing, NOT priority — measured: a "priority"
            # piece on the scalar ring finished ~5us late behind the sync
            # ring's bulk). Host packs zin to match the SBUF layout so
            # every descriptor line is 0.5-4KB contiguous.
            # piece a (the 128KB span 0 needs) goes out on the GpSimd
            # SWDGE ring, whose queue is idle at kernel start ~0.8us
            # before the sync sequencer finishes its preamble
            nc.gpsimd.dma_start(out=zbig[:, 0, :, 0:512],
                                in_=zin[:, 0, :, 0:512])
            for lo, hi in [(512, 1536), (1536, DBLK)]:
                nc.sync.dma_start(out=zbig[:, 0, :, lo:hi],
                                  in_=zin[:, 0, :, lo:hi])
            nc.sync.dma_start(out=zbig[:, 1, :, :], in_=zin[:, 1, :, :])
            nc.sync.dma_start(out=zbig[:, 2:4, :, :], in_=zin[:, 2:4, :, :])
            nc.sync.dma_start(out=auxt[:, :], in_=aux[:, :])

            def rhs_slice(x):
                """[x, x+512) of the rotated z^T, both chunks ([128,2,512])."""
                blk, off = divmod(x, DBLK)
                return zbig[:, blk, :, off:off + BLK]

            cs_ps = ps.tile([16, BLK], F32, tag="cs", bufs=1, name="cs_ps")
            ncs = 0  # colsum matmul counter for start/stop flags

            def cs_matmul(b, rhs):
                nonlocal ncs
                nc.tensor.matmul(
                    out=cs_ps,
                    lhsT=auxb[:, 144 - b:160 - b],
                    rhs=rhs,
                    start=(ncs == 0),
                    stop=(ncs == NCS_TOT - 1),
                )
                ncs += 1

            def emit_colsum(pending):
                for b, t2 in pending:
                    cs_matmul(b, t2)
                pending.clear()

            deferred = []

            etiles = {}
            t0s = {}
            pending = []
            for si, (lo, w, lblk) in enumerate(SPANS):
                cblocks = [
                    b for b in range(lo // BLK, (lo + w) // BLK)
                    if b not in SKIP_COLSUM.get(si, ())
                ]
                # last two spans skip the DVE tree and accumulate each E
                # row-tile straight into the colsum bank, interleaved with
                # the sim matmuls: a tree for them would finish after their
                # rt3 EXP and stall the in-order PE queue ~2.5us right at
                # the end (PE has plenty of slack for the extra matmuls).
                direct = si >= NSPAN - 2
                for rt in range(4):
                    psum = ps.tile([128, SW], F32, tag="sim", name="psum")
                    lhsT = zbig[:, lblk, :, rt * 128:(rt + 1) * 128]
                    for s in range(w // BLK):
                        nc.tensor.matmul(
                            out=psum[:, s * BLK:(s + 1) * BLK],
                            lhsT=lhsT,
                            rhs=rhs_slice(lo + s * BLK),
                            perf_mode=mybir.MatmulPerfMode.DoubleRow,
                        )
                    et = epool.tile([128, SW], BF16, tag=f"e{rt}",
                                    name=f"e{si}_{rt}")
                    etiles[(si, rt)] = et
                    nc.scalar.activation(
                        out=et[:, :w],
                        in_=psum[:, :w],
                        func=mybir.ActivationFunctionType.Exp,
                        scale=2.0,
                        accum_out=slots[:, si * 4 + rt:si * 4 + rt + 1],
                    )
                    # first tree level as soon as its inputs exist
                    if rt == 1 and not direct and cblocks:
                        t0 = epool.tile([128, SW], BF16, tag="t0",
                                        bufs=2, name="t0")
                        t0s[si] = t0
                        nc.vector.tensor_add(
                            t0[:, :w],
                            etiles[(si, 0)][:, :w],
                            etiles[(si, 1)][:, :w],
                        )
                    # release the previous span's column-sum matmuls at rt2
                    # so the in-order PE queue never waits on a DVE tree
                    # that finished only after the previous span's last EXP
                    if rt == 1:
                        emit_colsum(deferred)
                    if rt == 2:
                        emit_colsum(pending)
                    if direct and rt >= 1:
                        for b in cblocks:
                            off = b * BLK - lo
                            cs_matmul(b, etiles[(si, rt - 1)][:, off:off + BLK])
                if direct:
                    # the rt3-tail waits on this span's last EXP; emit it in
                    # the NEXT span's rt1 slot (or right here for the final
                    # span) so the PE queue doesn't bubble at the boundary
                    tail = [
                        (b, etiles[(si, 3)][:, b * BLK - lo:b * BLK - lo + BLK])
                        for b in cblocks
                    ]
                    if si == NSPAN - 1:
                        for b, t in tail:
                            cs_matmul(b, t)
                    else:
                        deferred.extend(tail)
                    continue
                if not cblocks:
                    continue
                # finish the add-tree for this span at full width
                t1 = epool.tile([128, SW], BF16, tag="t1", bufs=2, name="t1")
                t2 = epool.tile([128, SW], BF16, tag="t2", bufs=2, name="t2")
                nc.vector.tensor_add(
                    t1[:, :w],
                    etiles[(si, 2)][:, :w],
                    etiles[(si, 3)][:, :w],
                )
                nc.vector.tensor_add(t2[:, :w], t0s[si][:, :w], t1[:, :w])
                for b in cblocks:
                    off = b * BLK - lo
                    pending.append((b, t2[:, off:off + BLK]))

            assert ncs == NCS_TOT, ncs
            # tail: transpose row-sum slots on the PE via a bf16 identity
            # matmul ([128,24] -> [24,128]) so the output DMA uses >=512B
            # descriptors (bf16 quantization of the 24 per-row partials
            # costs ~1e-3 relative on S, far inside the 2e-2 budget), then
            # copy both PSUM results out on separate engines and DMA them
            # on separate HWDGE rings so the two chains run concurrently.
            nc.vector.tensor_copy(slots_b, slots)  # DVE is idle; scalar isn't
            st_ps = ps.tile([4 * NSPAN, 128], F32, tag="st", bufs=1,
                            name="st_ps")
            nc.tensor.matmul(out=st_ps, lhsT=slots_b, rhs=auxb[:, 0:128])
            nc.vector.tensor_copy(outcs, cs_ps)
            nc.scalar.copy(outst, st_ps)
            nc.sync.dma_start(out=out_cs[:, :], in_=outcs)
            nc.scalar.dma_start(out=out_st[:, :], in_=outst)

    nc.finalize()
    return nc


_NC_CACHE = None


def _get_nc():
    global _NC_CACHE
    if _NC_CACHE is None:
        _NC_CACHE = build_nc()
    return _NC_CACHE


def _normalize(zx, zy):
    z = np.concatenate(
        [np.asarray(zx, np.float32), np.asarray(zy, np.float32)], axis=0
    )
    zn = z / np.linalg.norm(z, axis=1, keepdims=True)
    return zn.astype(ml_dtypes.float8_e4m3fn)   # (N, D) fp8 e4m3


def _make_aux():
    i128 = np.eye(128, dtype=ml_dtypes.bfloat16)
    onep = np.zeros((128, 32), dtype=ml_dtypes.bfloat16)
    onep[:, 16] = 1.0
    raw = np.concatenate(
        [i128.view(np.uint8).reshape(128, 256),
         onep.view(np.uint8).reshape(128, 64)],
        axis=1,
    )
    return np.ascontiguousarray(raw).view(ml_dtypes.float8_e4m3fn)


_AUX = _make_aux()


def _make_in_maps(znb):
    ztb = np.ascontiguousarray(znb.T)           # (D, N)
    in_maps = []
    for c in range(NCORES):
        zt_c = np.roll(ztb, -BLK * c, axis=1)
        zp = np.empty((128, N // DBLK, 2, DBLK), dtype=znb.dtype)
        for ch in range(2):
            zp[:, :, ch, :] = zt_c[128 * ch:128 * (ch + 1)].reshape(
                128, N // DBLK, DBLK
            )
        in_maps.append({"zin": zp, "aux": _AUX})
    return in_maps


def run_device(zx, zy, **kwargs):
    """Run the 8-core kernel; returns (final scalar loss, BassKernelResults)."""
    nc = _get_nc()
    znb = _normalize(zx, zy)
    res = run_bass_kernel_spmd(
        nc, _make_in_maps(znb), core_ids=list(range(NCORES)), **kwargs
    )
    # ---- host combine (O(N) numpy) ----
    S = np.zeros(N, np.float64)
    for c in range(NCORES):
        _accumulate(S, res, c)

    znf = znb.astype(np.float32)
    selfdot = np.einsum("ij,ij->i", znf, znf, dtype=np.float64)
    posdot = np.einsum("ij,ij->i", znf, np.roll(znf, -B, axis=0),
                       dtype=np.float64)
    # sanity: each S_i sums 8192 exp values in [e^-2, e^2]; anything outside
    # a generous envelope means the device returned garbage
    ok = np.all(np.isfinite(S)) and np.all(S > 500.0) and np.all(S < 1e6)
    ce = np.log(S - np.exp(2.0 * selfdot)) - 2.0 * posdot
    pt = np.exp(-ce)
    focal = (1.0 - pt) ** GAMMA * ce
    loss = np.float32((ALPHA * focal).mean())
    if not ok:
        loss = np.float32(np.nan)
    return loss, res


def _accumulate(S, res, c):
    cs = np.asarray(res.results[c]["out_cs"], np.float64)    # [16, 512]
    st = np.asarray(res.results[c]["out_st"], np.float64)    # [4*NSPAN,128]
    for si in range(NSPAN):
        base = BLK * c + (B if SPANS[si][2] else 0)
        for rt in range(4):
            S[base + 128 * rt: base + 128 * (rt + 1)] += st[si * 4 + rt]
    for b in range(1, 16):
        base = (BLK * (c + b)) % N
        S[base:base + BLK] += cs[b]


def kernel(zx, zy):
    loss, _ = run_device(zx, zy)
    if not np.isfinite(loss):
        # very first execution of a freshly compiled NEFF has been observed
        # to produce garbage once (runtime warm-up); one retry is reliable
        loss, _ = run_device(zx, zy)
    return loss


if __name__ == "__main__":
    rng = np.random.default_rng(0)
    zx = rng.standard_normal((B, D), dtype=np.float32)
    zy = rng.standard_normal((B, D), dtype=np.float32)
    print(kernel(zx, zy))


# revision 18
# speedup vs baseline: 1.0186x; 1.0186x over previous
"""Balanced focal NT-Xent loss on 8 TRN2 NeuronCores — symmetric half-matrix.

The 8192x8192 similarity matrix S = zn zn^T is symmetric, so exp(2*S) is
computed ONCE per unordered 512x512 block pair. With 16 row-blocks of 512,
core c owns row-blocks A=c (col offsets +0..+8) and B=c+8 (offsets +0..+7):
exactly 17 of the 136 unique blocks per core, perfectly balanced, and the
positive-pair block (c, c+8) lands on core c's A strip.

Inputs are host-normalized (zn = z/|z| in fp32), quantized to fp8 e4m3
(halves DMA/SBUF; final rel err ~1e-5 vs the 2e-2 budget) and
column-rotated by -512c per core so every core runs one static SPMD
program over contiguous column spans: A covers [0:4608), B [4096:8192),
lhsT at [0:512) / [4096:4608). Both 128-row contraction chunks sit side by
side in one [128, 4, 2, 2048] tile so a single DoubleRow fp8 matmul
contracts all 256 rows — the kernel is then paced purely by the scalar
engine's fused exp (1 elem/cycle/lane at 1.2 GHz over 4.46M elements/core).

v2 changes vs baseline:
- Input arrives as ONE host-packed DRAM tensor matching the SBUF layout,
  loaded by 4 ordered HWDGE DMAs on the sync queue (block0's first 1536
  cols land first so span 0's matmuls can start ~2us in, vs ~11us when 12
  column-strided dma_starts fought over descriptor-gen slots). A small aux
  tensor carries a host-built I128 + one-hot column (bf16 bitcast in fp8
  bytes), killing the on-chip memsets.
- Column-sum add-tree runs once per SPAN at full width (15 DVE adds
  instead of 39 block-wise ones) — fewer instructions, fewer semaphores,
  shorter end-of-kernel semaphore-clear epilogue.
- Row-sum slots are PE-transposed ([128,24] -> [24,128]) and both outputs
  leave in one [24, 640] DMA with >=512B descriptors (the [128, 96B]
  output swarm cost ~1.8us in the baseline tail).

Row sums come free from the exp's accum_out; column sums (the transpose
side of each off-diagonal block) use the DVE add-tree (4 row-tiles -> 1,
bf16) plus a one-hot ones-matmul accumulating every block into one
[16, 512] PSUM bank (the final span skips the tree and accumulates its E
row-tiles directly so the post-exp tail stays short).

The host combines the per-core partial sums (O(N) numpy): S scatter-add,
self/positive dots, ce = ln(S - exp(2|q|^2)) - 2*pos, focal, mean.
"""

import sys

if "/opt/trn_rl_repo" not in sys.path:
    sys.path.insert(0, "/opt/trn_rl_repo")

import numpy as np
import ml_dtypes

import concourse.tile as tile
from concourse import bacc, mybir
from concourse.bass_utils import run_bass_kernel_spmd

B = 4096
D = 256
N = 2 * B          # 8192
NCORES = 8
BLK = 512          # symmetric block size
NBLK = N // BLK    # 16
TEMPERATURE = 0.5
GAMMA = 2.0
ALPHA = 0.25

BF16 = mybir.dt.bfloat16
FP8 = mybir.dt.float8e4
F32 = mybir.dt.float32

DBLK = 2048        # DMA column block
SW = 1536          # span tile width (3 PSUM banks)
AUXW = 320         # aux bytes/partition: I128 bf16 (256B) + one-hot col (64B)

# (col_lo, width, lhs_dma_block) for the span strips; spans 0-3 are the
# A row-block (own cols at [0:512)), spans 4-6 the B row-block ([4096:4608)).
# The first two spans are narrow so the first EXP gates on a 128KB DMA
# piece instead of 384KB — the exp train starts ~2.5us earlier, worth the
# +4 activations of fixed cost.
SPANS = [
    (0, 512, 0),
    (512, 1024, 0),
    (1536, 1536, 0),
    (3072, 1536, 0),
    (4096, 1536, 2),
    (5632, 1536, 2),
    (7168, 1024, 2),
]
NSPAN = len(SPANS)
# diag blocks (rowsum-only): rotated block 0 (A diag, in span 0) and
# block 8 (B diag, in span 4)
SKIP_COLSUM = {0: (0,), 4: (8,)}
NCS_TOT = 10 + 12 + 8  # 10 tree-reduced + 20 direct in the last two spans


def build_nc():
    nc = bacc.Bacc(None, target_bir_lowering=False)
    zin = nc.dram_tensor("zin", [128, N // DBLK, 2, DBLK], FP8,
                         kind="ExternalInput")
    aux = nc.dram_tensor("aux", [128, AUXW], FP8, kind="ExternalInput")
    out_cs = nc.dram_tensor("out_cs", [16, BLK], F32, kind="ExternalOutput")
    out_st = nc.dram_tensor("out_st", [4 * NSPAN, 128], F32,
                            kind="ExternalOutput")

    with tile.TileContext(nc) as tc:
        with (
            tc.tile_pool(name="big", bufs=1) as big,
            tc.tile_pool(name="epool", bufs=3) as epool,
            tc.tile_pool(name="ps", bufs=2, space="PSUM") as ps,
        ):
            # input z^T (normalized, rotated): 4 col-blocks, each holding
            # both 128-row contraction chunks side by side so a single
            # DoubleRow fp8 matmul contracts all 256 rows.
            zbig = big.tile([128, N // DBLK, 2, DBLK], FP8, tag="zbig",
                            name="zbig")
            auxt = big.tile([128, AUXW], FP8, tag="aux", name="auxt")
            auxb = auxt.bitcast(BF16)  # [:,0:128]=I128, [:,128:160]=one-hot
            slots = epool.tile([128, 4 * NSPAN], F32, tag="slots", bufs=1)
            slots_b = epool.tile([128, 4 * NSPAN], BF16, tag="slots_b", bufs=1)
            outcs = epool.tile([16, BLK], F32, tag="outcs", bufs=1)
            outst = epool.tile([4 * NSPAN, 128], F32, tag="outst", bufs=1)

            # All input DMAs ride the sync HWDGE ring, whose execution is
            # FIFO: pieces land in exactly consumption order, and the first
            # piece is the minimal 128KB span 0 needs (both rings feed the
            # same 16 SDMA engines round-robin, so a second ring gives
            # bandwidth sharing, NOT priority — measured: a "priority"
            # piece on the scalar ring finished ~5us late behind the sync
            # ring's bulk). Host packs zin to match the SBUF layout so
            # every descriptor line is 0.5-4KB contiguous.
            for lo, hi in [(0, 512), (512, 1536), (1536, DBLK)]:
                nc.sync.dma_start(out=zbig[:, 0, :, lo:hi],
                                  in_=zin[:, 0, :, lo:hi])
            nc.sync.dma_start(out=zbig[:, 1, :, :], in_=zin[:, 1, :, :])
            nc.sync.dma_start(out=zbig[:, 2:4, :, :], in_=zin[:, 2:4, :, :])
            # aux is needed only once colsums start (~15us in); the scalar
            # ring keeps it out of the sync FIFO ahead of blocks 2-3
            nc.scalar.dma_start(out=auxt[:, :], in_=aux[:, :])

            def rhs_slice(x):
                """[x, x+512) of the rotated z^T, both chunks ([128,2,512])."""
                blk, off = divmod(x, DBLK)
                return zbig[:, blk, :, off:off + BLK]

            cs_ps = ps.tile([16, BLK], F32, tag="cs", bufs=1, name="cs_ps")
            ncs = 0  # colsum matmul counter for start/stop flags

            def cs_matmul(b, rhs):
                nonlocal ncs
                nc.tensor.matmul(
                    out=cs_ps,
                    lhsT=auxb[:, 144 - b:160 - b],
                    rhs=rhs,
                    start=(ncs == 0),
                    stop=(ncs == NCS_TOT - 1),
                )
                ncs += 1

            def emit_colsum(pending):
                for b, t2 in pending:
                    cs_matmul(b, t2)
                pending.clear()

            deferred = []

            etiles = {}
            t0s = {}
            pending = []
            for si, (lo, w, lblk) in enumerate(SPANS):
                cblocks = [
                    b for b in range(lo // BLK, (lo + w) // BLK)
                    if b not in SKIP_COLSUM.get(si, ())
                ]
                # last two spans skip the DVE tree and accumulate each E
                # row-tile straight into the colsum bank, interleaved with
                # the sim matmuls: a tree for them would finish after their
                # rt3 EXP and stall the in-order PE queue ~2.5us right at
                # the end (PE has plenty of slack for the extra matmuls).
                direct = si >= NSPAN - 2
                for rt in range(4):
                    psum = ps.tile([128, SW], F32, tag="sim", name="psum")
                    lhsT = zbig[:, lblk, :, rt * 128:(rt + 1) * 128]
                    for s in range(w // BLK):
                        nc.tensor.matmul(
                            out=psum[:, s * BLK:(s + 1) * BLK],
                            lhsT=lhsT,
                            rhs=rhs_slice(lo + s * BLK),
                            perf_mode=mybir.MatmulPerfMode.DoubleRow,
                        )
                    et = epool.tile([128, SW], BF16, tag=f"e{rt}",
                                    name=f"e{si}_{rt}")
                    etiles[(si, rt)] = et
                    nc.scalar.activation(
                        out=et[:, :w],
                        in_=psum[:, :w],
                        func=mybir.ActivationFunctionType.Exp,
                        scale=2.0,
                        accum_out=slots[:, si * 4 + rt:si * 4 + rt + 1],
                    )
                    # first tree level as soon as its inputs exist
                    if rt == 1 and not direct and cblocks:
                        t0 = epool.tile([128, SW], BF16, tag="t0",
                                        bufs=2, name="t0")
                        t0s[si] = t0
                        nc.vector.tensor_add(
                            t0[:, :w],
                            etiles[(si, 0)][:, :w],
                            etiles[(si, 1)][:, :w],
                        )
                    # release the previous span's column-sum matmuls at rt2
                    # so the in-order PE queue never waits on a DVE tree
                    # that finished only after the previous span's last EXP
                    if rt == 1:
                        emit_colsum(deferred)
                    if rt == 2:
                        emit_colsum(pending)
                    if direct and rt >= 1:
                        for b in cblocks:
                            off = b * BLK - lo
                            cs_matmul(b, etiles[(si, rt - 1)][:, off:off + BLK])
                if direct:
                    # the rt3-tail waits on this span's last EXP; emit it in
                    # the NEXT span's rt1 slot (or right here for the final
                    # span) so the PE queue doesn't bubble at the boundary
                    tail = [
                        (b, etiles[(si, 3)][:, b * BLK - lo:b * BLK - lo + BLK])
                        for b in cblocks
                    ]
                    if si == NSPAN - 1:
                        for b, t in tail:
                            cs_matmul(b, t)
                    else:
                        deferred.extend(tail)
                    continue
                if not cblocks:
                    continue
                # finish the add-tree for this span at full width
                t1 = epool.tile([128, SW], BF16, tag="t1", bufs=2, name="t1")
                t2 = epool.tile([128, SW], BF16, tag="t2", bufs=2, name="t2")
                nc.vector.tensor_add(
                    t1[:, :w],
                    etiles[(si, 2)][:, :w],
                    etiles[(si, 3)][:, :w],
                )
                nc.vector.tensor_add(t2[:, :w], t0s[si][:, :w], t1[:, :w])
                for b in cblocks:
                    off = b * BLK - lo
                    pending.append((b, t2[:, off:off + BLK]))

            assert ncs == NCS_TOT, ncs
            # tail: transpose row-sum slots on the PE via a bf16 identity
            # matmul ([128,24] -> [24,128]) so the output DMA uses >=512B
            # descriptors (bf16 quantization of the 24 per-row partials
            # costs ~1e-3 relative on S, far inside the 2e-2 budget), then
            # copy both PSUM results out on separate engines and DMA them
            # on separate HWDGE rings so the two chains run concurrently.
            nc.vector.tensor_copy(slots_b, slots)  # DVE is idle; scalar isn't
            st_ps = ps.tile([4 * NSPAN, 128], F32, tag="st", bufs=1,
                            name="st_ps")
            nc.tensor.matmul(out=st_ps, lhsT=slots_b, rhs=auxb[:, 0:128])
            nc.vector.tensor_copy(outcs, cs_ps)
            nc.scalar.copy(outst, st_ps)
            nc.sync.dma_start(out=out_cs[:, :], in_=outcs)
            nc.scalar.dma_start(out=out_st[:, :], in_=outst)

    nc.finalize()
    return nc


_NC_CACHE = None


def _get_nc():
    global _NC_CACHE
    if _NC_CACHE is None:
        _NC_CACHE = build_nc()
    return _NC_CACHE


def _normalize(zx, zy):
    z = np.concatenate(
        [np.asarray(zx, np.float32), np.asarray(zy, np.float32)], axis=0
    )
    zn = z / np.linalg.norm(z, axis=1, keepdims=True)
    return zn.astype(ml_dtypes.float8_e4m3fn)   # (N, D) fp8 e4m3


def _make_aux():
    i128 = np.eye(128, dtype=ml_dtypes.bfloat16)
    onep = np.zeros((128, 32), dtype=ml_dtypes.bfloat16)
    onep[:, 16] = 1.0
    raw = np.concatenate(
        [i128.view(np.uint8).reshape(128, 256),
         onep.view(np.uint8).reshape(128, 64)],
        axis=1,
    )
    return np.ascontiguousarray(raw).view(ml_dtypes.float8_e4m3fn)


_AUX = _make_aux()


def _make_in_maps(znb):
    ztb = np.ascontiguousarray(znb.T)           # (D, N)
    in_maps = []
    for c in range(NCORES):
        zt_c = np.roll(ztb, -BLK * c, axis=1)
        zp = np.empty((128, N // DBLK, 2, DBLK), dtype=znb.dtype)
        for ch in range(2):
            zp[:, :, ch, :] = zt_c[128 * ch:128 * (ch + 1)].reshape(
                128, N // DBLK, DBLK
            )
        in_maps.append({"zin": zp, "aux": _AUX})
    return in_maps


def run_device(zx, zy, **kwargs):
    """Run the 8-core kernel; returns (final scalar loss, BassKernelResults)."""
    nc = _get_nc()
    znb = _normalize(zx, zy)
    res = run_bass_kernel_spmd(
        nc, _make_in_maps(znb), core_ids=list(range(NCORES)), **kwargs
    )
    # ---- host combine (O(N) numpy) ----
    S = np.zeros(N, np.float64)
    for c in range(NCORES):
        _accumulate(S, res, c)

    znf = znb.astype(np.float32)
    selfdot = np.einsum("ij,ij->i", znf, znf, dtype=np.float64)
    posdot = np.einsum("ij,ij->i", znf, np.roll(znf, -B, axis=0),
                       dtype=np.float64)
    # sanity: each S_i sums 8192 exp values in [e^-2, e^2]; anything outside
    # a generous envelope means the device returned garbage
    ok = np.all(np.isfinite(S)) and np.all(S > 500.0) and np.all(S < 1e6)
    ce = np.log(S - np.exp(2.0 * selfdot)) - 2.0 * posdot
    pt = np.exp(-ce)
    focal = (1.0 - pt) ** GAMMA * ce
    loss = np.float32((ALPHA * focal).mean())
    if not ok:
        loss = np.float32(np.nan)
    return loss, res


def _accumulate(S, res, c):
    cs = np.asarray(res.results[c]["out_cs"], np.float64)    # [16, 512]
    st = np.asarray(res.results[c]["out_st"], np.float64)    # [4*NSPAN,128]
    for si in range(NSPAN):
        base = BLK * c + (B if SPANS[si][2] else 0)
        for rt in range(4):
            S[base + 128 * rt: base + 128 * (rt + 1)] += st[si * 4 + rt]
    for b in range(1, 16):
        base = (BLK * (c + b)) % N
        S[base:base + BLK] += cs[b]


def kernel(zx, zy):
    loss, _ = run_device(zx, zy)
    if not np.isfinite(loss):
        # very first execution of a freshly compiled NEFF has been observed
        # to produce garbage once (runtime warm-up); one retry is reliable
        loss, _ = run_device(zx, zy)
    return loss


if __name__ == "__main__":
    rng = np.random.default_rng(0)
    zx = rng.standard_normal((B, D), dtype=np.float32)
    zy = rng.standard_normal((B, D), dtype=np.float32)
    print(kernel(zx, zy))
